# revision 24
# baseline (speedup 1.0000x reference)
"""Trainium2 Bass kernel for nn_DocREModel (segment_reduce, 8 cores).

Sharding: data-parallel. 4 docs x 800 pairs -> 8 cores, each core owns one
doc's half of the pairs (400). Small weights (Wh/Wt/Wb) replicated.

v2 design (all fp32; preds thresholding needs logits at ~1e-6):
  s1  indirect-DMA mention gather + logsumexp -> ent_emb [40,768]
  s2  one-hot matmul gathers -> hsT/tsT [768,400]
  s3  per (cc,h): gather-MMs -> pH/pT in PSUM, scalar copy of pT, DVE
      product, Sum_h via identity-matmul PSUM accumulation -> htS.
      Selector weights are 1.0 (not 1/3); epsilon scaled to 108e-5.
  s4  rs_un^T = seq^T @ htS; q = 1/(rowsum+108e-5) applied after the
      c-contraction (commutes); rsT = rs_un * q
  s5  extractors zh/zt (tanh chains), interleaved mc-wise with s6
  s6  block bilinear without SBUF mega-replication:
      partition p = b16*8 + c8 (b = bh*16+b16, c = ch*8+c8)
      zh-rep: 4 selector-matmuls per k -> PSUM -> scalar copy -> SBUF
      zt-rep: 8 gather + 4 doubling SBUF DMAs per k (1.64MB/k)
      products split DVE (batched broadcast-TT) + GPSIMD (plain TT)
      384 accumulating matmuls vs permuted Wb -> logits^T
  s7  preds on device (threshold compare + ones-matmul column count)
"""

import numpy as np

import concourse.bass as bass
import concourse.mybir as mybir
from concourse.tile import TileContext
from concourse.bass_utils import run_bass_kernel_spmd

F32 = mybir.dt.float32
F32R = mybir.dt.float32r
I32 = mybir.dt.int32
AF = mybir.ActivationFunctionType
OP = mybir.AluOpType

# problem shapes (hardcoded; kernel.py must be self-contained)
N_DOC, C, D, H = 4, 1024, 768, 12
E, M, P_DOC = 40, 3, 800
EMB, BLOCK, NREL = 768, 64, 97
NCORES = 8
P = P_DOC * N_DOC // NCORES          # 400 pairs per core
KB = EMB // BLOCK                    # 12 blocks
CC = C // 128                        # 8 c-chunks
DC = D // 128                        # 6 d-chunks
KC = 2 * D // 128                    # 12 contraction chunks for extractors
NJ = 32                              # (bh,ch) units per k
GP_UNITS = 10                        # ch-units per k on gpsimd (rest on DVE)

_CACHE = {}


def _split_multiwaits(nc, max_waits=1):
    """Walrus codegen accepts at most one sync-wait per instruction; hoist
    extras onto pure-wait InstEventSemaphore nops on the same engine."""
    nid = [0]
    f = nc.m.functions[0]
    for bb in f.blocks:
        insts = list(bb.instructions)
        out = []
        changed = False
        for inst in insts:
            si = getattr(inst, "sync_info", None)
            if si is not None and len(si.on_wait) > max_waits:
                waits = list(si.on_wait)
                extra, keep = waits[:-max_waits], waits[-max_waits:]
                for w in extra:
                    nid[0] += 1
                    ev = mybir.InstEventSemaphore(
                        name=f"W-{inst.name}-{nid[0]}", ins=[], outs=[])
                    ev.engine = inst.engine
                    ev.sync_info = mybir.SyncInfo(on_wait=[w], on_update=[])
                    out.append(ev)
                inst.sync_info = mybir.SyncInfo(on_wait=keep,
                                               on_update=list(si.on_update))
                changed = True
            out.append(inst)
        if changed:
            bb.instructions = out


def build_nc(split_waits=True):
    nc = bass.Bass()

    seq = nc.dram_tensor("seq", [C, D], F32, kind="ExternalInput")
    att_h = [nc.dram_tensor(f"att{h}", [C, C], F32R, kind="ExternalInput")
             for h in range(H)]
    posm = nc.dram_tensor("posm", [E, M], I32, kind="ExternalInput")
    posf = nc.dram_tensor("posf", [E * M, 1], I32, kind="ExternalInput")
    hsel3 = nc.dram_tensor("hsel3", [E * M, P], F32R, kind="ExternalInput")
    tsel3 = nc.dram_tensor("tsel3", [E * M, P], F32R, kind="ExternalInput")
    hsel1 = nc.dram_tensor("hsel1", [E, P], F32, kind="ExternalInput")
    tsel1 = nc.dram_tensor("tsel1", [E, P], F32, kind="ExternalInput")
    Whm = nc.dram_tensor("Whm", [128, DC, KC * 128], F32R, kind="ExternalInput")
    Wtm = nc.dram_tensor("Wtm", [128, DC, KC * 128], F32R, kind="ExternalInput")
    Wb = nc.dram_tensor("Wb", [KB, 128, NJ * NREL], F32R, kind="ExternalInput")
    bh = nc.dram_tensor("bh", [EMB], F32, kind="ExternalInput")
    bt = nc.dram_tensor("bt", [EMB], F32, kind="ExternalInput")
    bb = nc.dram_tensor("bb", [NREL], F32, kind="ExternalInput")
    seq_p = nc.dram_tensor("seq_p", [128, CC * D], F32R, kind="ExternalInput")
    ident = nc.dram_tensor("ident", [128, 128], F32R, kind="ExternalInput")
    onesr = nc.dram_tensor("onesr", [128, 1], F32R, kind="ExternalInput")
    selz = nc.dram_tensor("selz", [128, 4 * 128], F32R, kind="ExternalInput")
    logitsT_out = nc.dram_tensor("logitsT", [NREL, P], F32, kind="ExternalOutput")
    predsT_out = nc.dram_tensor("predsT", [NREL, P], F32, kind="ExternalOutput")

    with TileContext(nc) as tc:
        with tc.tile_pool(name="persist", bufs=1) as pp:
            # ---- constant / small loads ----
            pos_sb = pp.tile([E, M], I32)
            nc.sync.dma_start(pos_sb[:], posm[:, :])
            posf_sb = pp.tile([E * M, 1], I32)
            nc.sync.dma_start(posf_sb[:], posf[:, :])
            pact_cm = tc.tile_pool(name="acts", bufs=1)
            pact = pact_cm.__enter__()
            hsT = pact.tile([128, DC, P], F32R)
            tsT = pact.tile([128, DC, P], F32R)
            rsT = pact.tile([128, DC, P], F32R)
            psel_cm = tc.tile_pool(name="sels", bufs=1)
            psel = psel_cm.__enter__()
            hsel1_sb = psel.tile([E, P], F32)
            nc.sync.dma_start(hsel1_sb[:], hsel1[:, :])
            tsel1_sb = psel.tile([E, P], F32)
            nc.sync.dma_start(tsel1_sb[:], tsel1[:, :])
            hsel3_sb = psel.tile([E * M, P], F32R)
            nc.sync.dma_start(hsel3_sb[:], hsel3[:, :])
            tsel3_sb = psel.tile([E * M, P], F32R)
            nc.sync.dma_start(tsel3_sb[:], tsel3[:, :])
            bh_sb = pp.tile([128, DC], F32)
            nc.sync.dma_start(bh_sb[:], bh.rearrange("(c p) -> p c", p=128))
            bt_sb = pp.tile([128, DC], F32)
            nc.sync.dma_start(bt_sb[:], bt.rearrange("(c p) -> p c", p=128))
            bb_sb = pp.tile([NREL, 1], F32)
            nc.sync.dma_start(bb_sb[:], bb.rearrange("(r o) -> r o", o=1))
            ident_sb = pp.tile([128, 128], F32R)
            nc.sync.dma_start(ident_sb[:], ident[:, :])
            selz_sb = pp.tile([128, 4, 128], F32R)
            nc.sync.dma_start(selz_sb[:], selz.rearrange("q (b p) -> q b p", b=4))

            ones128 = pp.tile([128, 1], F32)
            nc.vector.memset(ones128[:], 1.0)
            ones128r = pp.tile([128, 1], F32R)
            nc.sync.dma_start(ones128r[:], onesr[:, :])
            ones1 = pp.tile([1, 128], F32)
            nc.vector.memset(ones1[:], 1.0)
            ones1r = pp.tile([1, NREL], F32)
            nc.vector.memset(ones1r[:], 1.0)
            ones97 = pp.tile([NREL, 1], F32)
            nc.vector.memset(ones97[:], 1.0)

            # persistent activations
            ent_emb = pp.tile([E, D], F32)
            zhT = pp.tile([128, DC, P], F32R)
            ztT = pp.tile([128, DC, P], F32R)
            logits_sb = pp.tile([NREL, P], F32)
            preds_sb = pp.tile([NREL, P], F32)

            # ---- stage 1: mention gather + logsumexp -> ent_emb ----
            with tc.tile_pool(name="lse", bufs=1) as pl:
                m_emb = pl.tile([E, M, D], F32)
                for m in range(M):
                    nc.gpsimd.indirect_dma_start(
                        out=m_emb[:, m, :], out_offset=None,
                        in_=seq[:, :],
                        in_offset=bass.IndirectOffsetOnAxis(
                            ap=pos_sb[:, m:m + 1], axis=0),
                    )
                mx = pl.tile([E, D], F32)
                nc.vector.tensor_max(mx[:], m_emb[:, 0, :], m_emb[:, 1, :])
                nc.vector.tensor_max(mx[:], mx[:], m_emb[:, 2, :])
                ssum = pl.tile([E, D], F32)
                for m in range(M):
                    dm = pl.tile([E, D], F32, tag="dm")
                    nc.vector.tensor_sub(dm[:], m_emb[:, m, :], mx[:])
                    em = pl.tile([E, D], F32, tag="em")
                    nc.scalar.activation(em[:], dm[:], AF.Exp)
                    if m == 0:
                        nc.vector.tensor_copy(ssum[:], em[:])
                    else:
                        nc.vector.tensor_add(ssum[:], ssum[:], em[:])
                nc.scalar.activation(ssum[:], ssum[:], AF.Ln)
                nc.vector.tensor_add(ent_emb[:], ssum[:], mx[:])

            # ---- stage 2: hs^T / ts^T via one-hot matmuls ----
            with tc.tile_pool(name="g2", bufs=2, space="PSUM") as ps2p:
                for mc in range(DC):
                    ps = ps2p.tile([128, P], F32, tag="gather")
                    nc.tensor.matmul(ps[:], lhsT=ent_emb[:, mc * 128:(mc + 1) * 128],
                                     rhs=hsel1_sb[:], start=True, stop=True)
                    nc.scalar.copy(hsT[:, mc, :], ps[:])
                    psb = ps2p.tile([128, P], F32, tag="gather")
                    nc.tensor.matmul(psb[:], lhsT=ent_emb[:, mc * 128:(mc + 1) * 128],
                                     rhs=tsel1_sb[:], start=True, stop=True)
                    nc.scalar.copy(tsT[:, mc, :], psb[:])

            # ---- stage 3: attention gather + ht via identity-MM h-accum ----
            ps34_cm = tc.tile_pool(name="s34", bufs=1)
            ps34 = ps34_cm.__enter__()
            htS = ps34.tile([128, CC, P], F32R)
            qb = ps34.tile([128, P], F32)
            with tc.tile_pool(name="araw", bufs=1) as pa, \
                 tc.tile_pool(name="htp", bufs=3, space="PSUM") as psH, \
                 tc.tile_pool(name="hta", bufs=2, space="PSUM") as psA, \
                 tc.tile_pool(name="httmp", bufs=4) as pt:
                araw = pa.tile([E * M, H, C], F32R)
                for h in range(H):
                    nc.gpsimd.indirect_dma_start(
                        out=araw[:, h, :], out_offset=None,
                        in_=att_h[h][:, :],
                        in_offset=bass.IndirectOffsetOnAxis(
                            ap=posf_sb[:, 0:1], axis=0),
                    )
                LAG = 2
                for cc in range(CC):
                    ht_ps = psA.tile([128, P], F32, tag="htacc")
                    prods = []
                    for h in range(H + LAG):
                        if h < H:
                            pH = psH.tile([128, P], F32, tag="ph")
                            nc.tensor.matmul(
                                pH[:],
                                lhsT=araw[:, h, cc * 128:(cc + 1) * 128],
                                rhs=hsel3_sb[:],
                                start=True, stop=True)
                            pT = psH.tile([128, P], F32, tag="pt")
                            nc.tensor.matmul(
                                pT[:],
                                lhsT=araw[:, h, cc * 128:(cc + 1) * 128],
                                rhs=tsel3_sb[:],
                                start=True, stop=True)
                            sT = pt.tile([128, P], F32, tag="st")
                            nc.scalar.copy(sT[:], pT[:])
                            prod = pt.tile([128, P], F32R, tag="prod")
                            nc.vector.tensor_mul(prod[:], pH[:], sT[:])
                            prods.append(prod)
                        if h >= LAG:
                            ha = h - LAG
                            nc.tensor.matmul(
                                ht_ps[:], lhsT=ident_sb[:],
                                rhs=prods[ha][:],
                                start=(ha == 0), stop=(ha == H - 1))
                    nc.scalar.copy(htS[:, cc, :], ht_ps[:])

            # normalizer q (applied after s4's c-contraction)
            with tc.tile_pool(name="nrm", bufs=1, space="PSUM") as psN, \
                 tc.tile_pool(name="nrmt", bufs=1) as pnt:
                psR = psN.tile([1, P], F32, tag="rowsum")
                for cc in range(CC):
                    nc.tensor.matmul(psR[:], lhsT=ones128r[:],
                                     rhs=htS[:, cc, :],
                                     start=(cc == 0), stop=(cc == CC - 1))
                q = pnt.tile([1, P], F32, tag="q")
                nc.vector.tensor_scalar_add(q[:], psR[:], 108.0e-5)
                qr = pnt.tile([1, P], F32, tag="qr")
                nc.vector.reciprocal(qr[:], q[:])
                psQ = psN.tile([128, P], F32, tag="qb")
                nc.tensor.matmul(psQ[:], lhsT=ones1[:], rhs=qr[:],
                                 start=True, stop=True)
                nc.scalar.copy(qb[:], psQ[:])

            # ---- stage 4: rs^T = (seq^T @ htS) * q ----
            with tc.tile_pool(name="seqp", bufs=1) as psq, \
                 tc.tile_pool(name="rsps", bufs=2, space="PSUM") as psRS:
                seq_sb = psq.tile([128, CC, D], F32R)
                nc.sync.dma_start(seq_sb[:], seq_p.rearrange("p (c d) -> p c d", c=CC))
                for mc in range(DC):
                    ps = psRS.tile([128, P], F32, tag="rs")
                    for cc in range(CC):
                        nc.tensor.matmul(
                            ps[:],
                            lhsT=seq_sb[:, cc, mc * 128:(mc + 1) * 128],
                            rhs=htS[:, cc, :],
                            start=(cc == 0), stop=(cc == CC - 1))
                    nc.vector.tensor_mul(rsT[:, mc, :], ps[:], qb[:])
            ps34_cm.__exit__(None, None, None)
            psel_cm.__exit__(None, None, None)

            # ---- stages 5+6 interleaved over mc / k ----
            dve_units = NJ - GP_UNITS  # per k, from unit 0
            psL_cm = tc.tile_pool(name="lg", bufs=1, space="PSUM")
            psL = psL_cm.__enter__()
            logits_ps = psL.tile([NREL, P], F32)
            with tc.tile_pool(name="wexp", bufs=2) as pwx, \
                 tc.tile_pool(name="wbs", bufs=2) as pwb, \
                 tc.tile_pool(name="ztr", bufs=2) as pzt, \
                 tc.tile_pool(name="zhc", bufs=2) as pzh, \
                 tc.tile_pool(name="otp", bufs=1) as pot, \
                 tc.tile_pool(name="exps", bufs=2, space="PSUM") as psE, \
                 tc.tile_pool(name="zrp", bufs=1, space="PSUM") as psZ:
                for mc in range(DC):
                    # s5 chunk mc: zh then zt
                    wh_mc = pwx.tile([128, KC, 128], F32R, tag="whm")
                    nc.sync.dma_start(wh_mc[:], Whm[:, mc].rearrange(
                        "p (k j) -> p k j", k=KC))
                    wt_mc = pwx.tile([128, KC, 128], F32R, tag="wtm")
                    nc.sync.dma_start(wt_mc[:], Wtm[:, mc].rearrange(
                        "p (k j) -> p k j", k=KC))
                    ps = psE.tile([128, P], F32, tag="ex")
                    for kc in range(KC):
                        rhs = hsT[:, kc, :] if kc < DC else rsT[:, kc - DC, :]
                        nc.tensor.matmul(
                            ps[:], lhsT=wh_mc[:, kc, :],
                            rhs=rhs, start=(kc == 0), stop=(kc == KC - 1))
                    nc.scalar.activation(zhT[:, mc, :], ps[:], AF.Tanh,
                                         bias=bh_sb[:, mc:mc + 1])
                    nc.scalar.activation(zhT[:, mc, :], zhT[:, mc, :], AF.Tanh)
                    psb = psE.tile([128, P], F32, tag="ex")
                    for kc in range(KC):
                        rhs = tsT[:, kc, :] if kc < DC else rsT[:, kc - DC, :]
                        nc.tensor.matmul(
                            psb[:], lhsT=wt_mc[:, kc, :],
                            rhs=rhs, start=(kc == 0), stop=(kc == KC - 1))
                    nc.scalar.activation(ztT[:, mc, :], psb[:], AF.Tanh,
                                         bias=bt_sb[:, mc:mc + 1])

                    # s6 for k = 2mc, 2mc+1
                    # zt-rep chains for both k first (scalar HWDGE ring)
                    ztreps = {}
                    for k in (2 * mc, 2 * mc + 1):
                        kc2, hf = k // 2, k % 2
                        p0 = hf * 64
                        ztrep = pzt.tile([128, CC, P], F32R, tag="ztr")
                        for ch in range(8):
                            nc.scalar.dma_start(
                                ztrep[0:8, ch, :],
                                ztT[p0 + ch * 8:p0 + ch * 8 + 8, kc2, :])
                        s = 8
                        while s < 128:
                            nc.scalar.dma_start(ztrep[s:2 * s, :, :],
                                                ztrep[0:s, :, :])
                            s *= 2
                        ztreps[k] = ztrep
                    for k in (2 * mc, 2 * mc + 1):
                        kc2, hf = k // 2, k % 2
                        p0 = hf * 64
                        ztrep = ztreps[k]
                        wb_k = pwb.tile([128, NJ, NREL], F32R, tag="wbk")
                        nc.sync.dma_start(
                            wb_k[:], Wb[k].rearrange("p (j r) -> p j r", j=NJ))
                        # zh-rep: 4 selector matmuls + scalar copies
                        zhcs = []
                        for bh2 in range(4):
                            zr = psZ.tile([128, P], F32, tag=f"zr{bh2}")
                            nc.tensor.matmul(
                                zr[:], lhsT=selz_sb[p0:p0 + 64, bh2, :],
                                rhs=zhT[p0:p0 + 64, kc2, :],
                                start=True, stop=True)
                            zc = pzh.tile([128, P], F32R, tag=f"zhc{bh2}")
                            nc.scalar.copy(zc[:], zr[:])
                            zhcs.append(zc)
                        # products: DVE bh 0-2 (batched), GPSIMD bh 3 (one op)
                        for bh2 in range(4):
                            zcb = zhcs[bh2][:, :].rearrange(
                                "p (o n) -> p o n", o=1)
                            o_t = pot.tile([128, 8, P], F32R, tag=f"ot{bh2}")
                            if bh2 < 3:
                                nc.vector.tensor_tensor(
                                    o_t[:, :, :], ztrep[:, bh2 * 0:8, :][:, 0:8, :],
                                    zcb.to_broadcast([128, 8, P]),
                                    op=OP.mult)
                            else:
                                nc.gpsimd.tensor_tensor(
                                    o_t[:, :, :], ztrep[:, 0:8, :],
                                    zcb.to_broadcast([128, 8, P]),
                                    op=OP.mult)
                            for ch in range(8):
                                j = bh2 * 8 + ch
                                nc.tensor.matmul(
                                    logits_ps[:],
                                    lhsT=wb_k[:, j, :],
                                    rhs=o_t[:, ch, :],
                                    start=(k == 0 and j == 0),
                                    stop=(k == KB - 1 and j == NJ - 1))

            # ---- stage 7: bias, preds, write out ----
            with tc.tile_pool(name="fin", bufs=1, space="PSUM") as psF:
                nc.vector.tensor_scalar_add(logits_sb[:], logits_ps[:],
                                            bb_sb[:, 0:1])
                nc.sync.dma_start(logitsT_out[:, :], logits_sb[:])
                psTh = psF.tile([NREL, P], F32, tag="th")
                nc.tensor.matmul(psTh[:], lhsT=ones1r[:],
                                 rhs=logits_sb[0:1, :], start=True, stop=True)
                nc.vector.tensor_tensor(preds_sb[:], logits_sb[:], psTh[:],
                                        op=OP.is_gt)
                psCt = psF.tile([1, P], F32, tag="cnt")
                nc.tensor.matmul(psCt[:], lhsT=ones97[:],
                                 rhs=preds_sb[:, :], start=True, stop=True)
                nc.vector.tensor_single_scalar(preds_sb[0:1, :], psCt[:],
                                               0.0, OP.is_equal)
                nc.sync.dma_start(predsT_out[:, :], preds_sb[:])
            psL_cm.__exit__(None, None, None)
            pact_cm.__exit__(None, None, None)

    if split_waits:
        _split_multiwaits(nc)
    nc.finalize()
    return nc


def _permute_wb(Wb):
    """Wb rows (k, b, c) -> [k, part=b16*8+c8, j=(bh*8+ch), r] with
    b = bh*16+b16, c = ch*8+c8."""
    W = np.asarray(Wb, np.float32).reshape(KB, 4, 16, 8, 8, NREL)
    W = W.transpose(0, 2, 4, 1, 3, 5)            # k, b16, c8, bh, ch, r
    return np.ascontiguousarray(W.reshape(KB, 128, NJ * NREL))


def _permute_wx(W):
    """[1536, 768] -> [128, mc, kc*128] column-chunked for streaming."""
    W = np.asarray(W, np.float32).reshape(KC, 128, DC, 128)
    W = W.transpose(1, 2, 0, 3)                  # p, mc, kc, j
    return np.ascontiguousarray(W.reshape(128, DC, KC * 128))


def _make_selz():
    s = np.zeros((128, 4, 128), np.float32)
    for bh2 in range(4):
        for part in range(128):
            s[bh2 * 16 + part // 8, bh2, part] = 1.0          # half 0 rows
            s[64 + bh2 * 16 + part // 8, bh2, part] = 1.0     # half 1 rows
    return np.ascontiguousarray(s.reshape(128, 4 * 128))


def _make_inputs(core, sequence_output, attention, mention_pos, hts,
                 Wh, bh, Wt, bt, Wb, bb):
    d = core // 2
    half = core % 2
    pos = (np.asarray(mention_pos[d]) + 1).astype(np.int32)      # [E, M]
    ht = np.asarray(hts[d][half * P:(half + 1) * P]).astype(np.int64)  # [P,2]
    h_idx, t_idx = ht[:, 0], ht[:, 1]

    hsel1 = np.zeros((E, P), np.float32)
    hsel1[h_idx, np.arange(P)] = 1.0
    tsel1 = np.zeros((E, P), np.float32)
    tsel1[t_idx, np.arange(P)] = 1.0
    hsel3 = np.zeros((E * M, P), np.float32)
    tsel3 = np.zeros((E * M, P), np.float32)
    for m in range(M):
        hsel3[h_idx * M + m, np.arange(P)] = 1.0
        tsel3[t_idx * M + m, np.arange(P)] = 1.0

    seq_d = np.asarray(sequence_output[d], np.float32)
    im = {
        "seq": np.ascontiguousarray(seq_d),
        "seq_p": np.ascontiguousarray(
            seq_d.reshape(CC, 128, D).transpose(1, 0, 2).reshape(128, CC * D)),
        "posm": pos,
        "posf": np.ascontiguousarray(pos.reshape(E * M, 1)),
        "hsel1": hsel1, "tsel1": tsel1, "hsel3": hsel3, "tsel3": tsel3,
        "Whm": _CACHE.setdefault("Whm", _permute_wx(Wh)),
        "Wtm": _CACHE.setdefault("Wtm", _permute_wx(Wt)),
        "Wb": _CACHE.setdefault("Wbp", _permute_wb(Wb)),
        "bh": np.ascontiguousarray(bh, np.float32),
        "bt": np.ascontiguousarray(bt, np.float32),
        "bb": np.ascontiguousarray(bb, np.float32),
        "ident": _CACHE.setdefault(
            "ident", np.ascontiguousarray(np.eye(128, dtype=np.float32))),
        "onesr": _CACHE.setdefault(
            "onesr", np.ones((128, 1), np.float32)),
        "selz": _CACHE.setdefault("selz", _make_selz()),
    }
    for h in range(H):
        im[f"att{h}"] = np.ascontiguousarray(attention[d, h], np.float32)
    return im


LAST_RESULTS = None


def kernel(sequence_output, attention, mention_pos, hts,
           Wh, bh, Wt, bt, Wb, bb):
    global LAST_RESULTS
    if "nc" not in _CACHE:
        _CACHE["nc"] = build_nc()
    nc = _CACHE["nc"]

    in_maps = [_make_inputs(c, sequence_output, attention, mention_pos, hts,
                            Wh, bh, Wt, bt, Wb, bb) for c in range(NCORES)]
    res = run_bass_kernel_spmd(nc, in_maps, core_ids=list(range(NCORES)))
    LAST_RESULTS = res

    logits = np.concatenate(
        [np.ascontiguousarray(r["logitsT"].T) for r in res.results], axis=0)
    preds = np.concatenate(
        [np.ascontiguousarray(r["predsT"].T) for r in res.results], axis=0)
    return logits.astype(np.float32), preds.astype(np.float32)


# revision 25
# speedup vs baseline: 1.1566x; 1.1566x over previous
"""Trainium2 Bass kernel for nn_DocREModel (segment_reduce, 8 cores).

Sharding: data-parallel. 4 docs x 800 pairs -> 8 cores, each core owns one
doc's half of the pairs (400). Small weights (Wh/Wt/Wb) replicated.

v2 design (all fp32; preds thresholding needs logits at ~1e-6):
  s1  indirect-DMA mention gather + logsumexp -> ent_emb [40,768]
  s2  one-hot matmul gathers -> hsT/tsT [768,400]
  s3  per (cc,h): gather-MMs -> pH/pT in PSUM, scalar copy of pT, DVE
      product, Sum_h via identity-matmul PSUM accumulation -> htS.
      Selector weights are 1.0 (not 1/3); epsilon scaled to 108e-5.
  s4  rs_un^T = seq^T @ htS; q = 1/(rowsum+108e-5) applied after the
      c-contraction (commutes); rsT = rs_un * q
  s5  extractors zh/zt (tanh chains), interleaved mc-wise with s6
  s6  block bilinear without SBUF mega-replication:
      partition p = b16*8 + c8 (b = bh*16+b16, c = ch*8+c8)
      zh-rep: 4 selector-matmuls per k -> PSUM -> scalar copy -> SBUF
      zt-rep: 8 gather + 4 doubling SBUF DMAs per k (1.64MB/k)
      products split DVE (batched broadcast-TT) + GPSIMD (plain TT)
      384 accumulating matmuls vs permuted Wb -> logits^T
  s7  preds on device (threshold compare + ones-matmul column count)
"""

import numpy as np

import concourse.bass as bass
import concourse.mybir as mybir
from concourse.tile import TileContext
from concourse.bass_utils import run_bass_kernel_spmd

F32 = mybir.dt.float32
F32R = mybir.dt.float32r
I32 = mybir.dt.int32
AF = mybir.ActivationFunctionType
OP = mybir.AluOpType

# problem shapes (hardcoded; kernel.py must be self-contained)
N_DOC, C, D, H = 4, 1024, 768, 12
E, M, P_DOC = 40, 3, 800
EMB, BLOCK, NREL = 768, 64, 97
NCORES = 8
P = P_DOC * N_DOC // NCORES          # 400 pairs per core
KB = EMB // BLOCK                    # 12 blocks
CC = C // 128                        # 8 c-chunks
DC = D // 128                        # 6 d-chunks
KC = 2 * D // 128                    # 12 contraction chunks for extractors
NJ = 32                              # (bh,ch) units per k
GP_UNITS = 10                        # ch-units per k on gpsimd (rest on DVE)

_CACHE = {}


def _split_multiwaits(nc, max_waits=1):
    """Walrus codegen accepts at most one sync-wait per instruction; hoist
    extras onto pure-wait InstEventSemaphore nops on the same engine."""
    nid = [0]
    f = nc.m.functions[0]
    for bb in f.blocks:
        insts = list(bb.instructions)
        out = []
        changed = False
        for inst in insts:
            si = getattr(inst, "sync_info", None)
            if si is not None and len(si.on_wait) > max_waits:
                waits = list(si.on_wait)
                extra, keep = waits[:-max_waits], waits[-max_waits:]
                for w in extra:
                    nid[0] += 1
                    ev = mybir.InstEventSemaphore(
                        name=f"W-{inst.name}-{nid[0]}", ins=[], outs=[])
                    ev.engine = inst.engine
                    ev.sync_info = mybir.SyncInfo(on_wait=[w], on_update=[])
                    out.append(ev)
                inst.sync_info = mybir.SyncInfo(on_wait=keep,
                                               on_update=list(si.on_update))
                changed = True
            out.append(inst)
        if changed:
            bb.instructions = out


def build_nc(split_waits=True):
    nc = bass.Bass()

    seq = nc.dram_tensor("seq", [C, D], F32, kind="ExternalInput")
    att_h = [nc.dram_tensor(f"att{h}", [C, C], F32R, kind="ExternalInput")
             for h in range(H)]
    posm = nc.dram_tensor("posm", [E, M], I32, kind="ExternalInput")
    posf = nc.dram_tensor("posf", [E * M, 1], I32, kind="ExternalInput")
    hsel3 = nc.dram_tensor("hsel3", [E * M, P], F32R, kind="ExternalInput")
    tsel3 = nc.dram_tensor("tsel3", [E * M, P], F32R, kind="ExternalInput")
    hsel1 = nc.dram_tensor("hsel1", [E, P], F32, kind="ExternalInput")
    tsel1 = nc.dram_tensor("tsel1", [E, P], F32, kind="ExternalInput")
    Whm = nc.dram_tensor("Whm", [128, DC, KC * 128], F32R, kind="ExternalInput")
    Wtm = nc.dram_tensor("Wtm", [128, DC, KC * 128], F32R, kind="ExternalInput")
    Wb = nc.dram_tensor("Wb", [KB, 128, NJ * NREL], F32R, kind="ExternalInput")
    bh = nc.dram_tensor("bh", [EMB], F32, kind="ExternalInput")
    bt = nc.dram_tensor("bt", [EMB], F32, kind="ExternalInput")
    bb = nc.dram_tensor("bb", [NREL], F32, kind="ExternalInput")
    seq_p = nc.dram_tensor("seq_p", [128, CC * D], F32R, kind="ExternalInput")
    ident = nc.dram_tensor("ident", [128, 128], F32R, kind="ExternalInput")
    onesr = nc.dram_tensor("onesr", [128, 1], F32R, kind="ExternalInput")
    selz = nc.dram_tensor("selz", [128, 4 * 128], F32R, kind="ExternalInput")
    logitsT_out = nc.dram_tensor("logitsT", [NREL, P], F32, kind="ExternalOutput")
    predsT_out = nc.dram_tensor("predsT", [NREL, P], F32, kind="ExternalOutput")

    with TileContext(nc) as tc:
        with tc.tile_pool(name="persist", bufs=1) as pp:
            # ---- constant / small loads ----
            pos_sb = pp.tile([E, M], I32)
            nc.sync.dma_start(pos_sb[:], posm[:, :])
            posf_sb = pp.tile([E * M, 1], I32)
            nc.sync.dma_start(posf_sb[:], posf[:, :])
            pact_cm = tc.tile_pool(name="acts", bufs=1)
            pact = pact_cm.__enter__()
            hsT = pact.tile([128, DC, P], F32R)
            tsT = pact.tile([128, DC, P], F32R)
            rsT = pact.tile([128, DC, P], F32R)
            psel_cm = tc.tile_pool(name="sels", bufs=1)
            psel = psel_cm.__enter__()
            hsel1_sb = psel.tile([E, P], F32)
            nc.sync.dma_start(hsel1_sb[:], hsel1[:, :])
            tsel1_sb = psel.tile([E, P], F32)
            nc.sync.dma_start(tsel1_sb[:], tsel1[:, :])
            hsel3_sb = psel.tile([E * M, P], F32R)
            nc.sync.dma_start(hsel3_sb[:], hsel3[:, :])
            tsel3_sb = psel.tile([E * M, P], F32R)
            nc.sync.dma_start(tsel3_sb[:], tsel3[:, :])
            bh_sb = pp.tile([128, DC], F32)
            nc.sync.dma_start(bh_sb[:], bh.rearrange("(c p) -> p c", p=128))
            bt_sb = pp.tile([128, DC], F32)
            nc.sync.dma_start(bt_sb[:], bt.rearrange("(c p) -> p c", p=128))
            bb_sb = pp.tile([NREL, 1], F32)
            nc.sync.dma_start(bb_sb[:], bb.rearrange("(r o) -> r o", o=1))
            ident_sb = pp.tile([128, 128], F32R)
            nc.sync.dma_start(ident_sb[:], ident[:, :])
            selz_sb = pp.tile([128, 4, 128], F32R)
            nc.sync.dma_start(selz_sb[:], selz.rearrange("q (b p) -> q b p", b=4))

            # PE pre-warm: dense dummy matmuls while input DMAs land,
            # so HAM reaches K=8/8 before real compute begins.
            with tc.tile_pool(name="warm", bufs=1, space="PSUM") as pwarm:
                wps = pwarm.tile([128, 128], F32, tag="w")
                for _ in range(80):
                    nc.tensor.matmul(wps[:], lhsT=ident_sb[:],
                                     rhs=selz_sb[:, 0, :], start=True,
                                     stop=True)

            ones128 = pp.tile([128, 1], F32)
            nc.vector.memset(ones128[:], 1.0)
            ones128r = pp.tile([128, 1], F32R)
            nc.sync.dma_start(ones128r[:], onesr[:, :])
            ones1 = pp.tile([1, 128], F32)
            nc.vector.memset(ones1[:], 1.0)
            ones1r = pp.tile([1, NREL], F32)
            nc.vector.memset(ones1r[:], 1.0)
            ones97 = pp.tile([NREL, 1], F32)
            nc.vector.memset(ones97[:], 1.0)

            # persistent activations
            ent_emb = pp.tile([E, D], F32)
            zhT = pp.tile([128, DC, P], F32R)
            ztT = pp.tile([128, DC, P], F32R)
            logits_sb = pp.tile([NREL, P], F32)
            preds_sb = pp.tile([NREL, P], F32)

            # ---- stage 1: mention gather + logsumexp -> ent_emb ----
            with tc.tile_pool(name="lse", bufs=1) as pl:
                m_emb = pl.tile([E, M, D], F32)
                for m in range(M):
                    nc.gpsimd.indirect_dma_start(
                        out=m_emb[:, m, :], out_offset=None,
                        in_=seq[:, :],
                        in_offset=bass.IndirectOffsetOnAxis(
                            ap=pos_sb[:, m:m + 1], axis=0),
                    )
                mx = pl.tile([E, D], F32)
                nc.vector.tensor_max(mx[:], m_emb[:, 0, :], m_emb[:, 1, :])
                nc.vector.tensor_max(mx[:], mx[:], m_emb[:, 2, :])
                ssum = pl.tile([E, D], F32)
                for m in range(M):
                    dm = pl.tile([E, D], F32, tag="dm")
                    nc.vector.tensor_sub(dm[:], m_emb[:, m, :], mx[:])
                    em = pl.tile([E, D], F32, tag="em")
                    nc.scalar.activation(em[:], dm[:], AF.Exp)
                    if m == 0:
                        nc.vector.tensor_copy(ssum[:], em[:])
                    else:
                        nc.vector.tensor_add(ssum[:], ssum[:], em[:])
                nc.scalar.activation(ssum[:], ssum[:], AF.Ln)
                nc.vector.tensor_add(ent_emb[:], ssum[:], mx[:])

            # ---- stage 2: hs^T / ts^T via one-hot matmuls ----
            with tc.tile_pool(name="g2", bufs=2, space="PSUM") as ps2p:
                for mc in range(DC):
                    ps = ps2p.tile([128, P], F32, tag="gather")
                    nc.tensor.matmul(ps[:], lhsT=ent_emb[:, mc * 128:(mc + 1) * 128],
                                     rhs=hsel1_sb[:], start=True, stop=True)
                    nc.scalar.copy(hsT[:, mc, :], ps[:])
                    psb = ps2p.tile([128, P], F32, tag="gather")
                    nc.tensor.matmul(psb[:], lhsT=ent_emb[:, mc * 128:(mc + 1) * 128],
                                     rhs=tsel1_sb[:], start=True, stop=True)
                    nc.scalar.copy(tsT[:, mc, :], psb[:])

            # ---- stage 3: attention gather + ht via identity-MM h-accum ----
            ps34_cm = tc.tile_pool(name="s34", bufs=1)
            ps34 = ps34_cm.__enter__()
            htS = ps34.tile([128, CC, P], F32R)
            qb = ps34.tile([128, P], F32)
            with tc.tile_pool(name="araw", bufs=1) as pa, \
                 tc.tile_pool(name="htp", bufs=3, space="PSUM") as psH, \
                 tc.tile_pool(name="hta", bufs=2, space="PSUM") as psA, \
                 tc.tile_pool(name="httmp", bufs=4) as pt:
                araw = pa.tile([E * M, H, C], F32R)
                for h in range(H):
                    nc.gpsimd.indirect_dma_start(
                        out=araw[:, h, :], out_offset=None,
                        in_=att_h[h][:, :],
                        in_offset=bass.IndirectOffsetOnAxis(
                            ap=posf_sb[:, 0:1], axis=0),
                    )
                LAG = 2
                for cc in range(CC):
                    ht_ps = psA.tile([128, P], F32, tag="htacc")
                    prods = []
                    for h in range(H + LAG):
                        if h < H:
                            pH = psH.tile([128, P], F32, tag="ph")
                            nc.tensor.matmul(
                                pH[:],
                                lhsT=araw[:, h, cc * 128:(cc + 1) * 128],
                                rhs=hsel3_sb[:],
                                start=True, stop=True)
                            pT = psH.tile([128, P], F32, tag="pt")
                            nc.tensor.matmul(
                                pT[:],
                                lhsT=araw[:, h, cc * 128:(cc + 1) * 128],
                                rhs=tsel3_sb[:],
                                start=True, stop=True)
                            sT = pt.tile([128, P], F32, tag="st")
                            nc.scalar.copy(sT[:], pT[:])
                            prod = pt.tile([128, P], F32R, tag="prod")
                            nc.vector.tensor_mul(prod[:], pH[:], sT[:])
                            prods.append(prod)
                        if h >= LAG:
                            ha = h - LAG
                            nc.tensor.matmul(
                                ht_ps[:], lhsT=ident_sb[:],
                                rhs=prods[ha][:],
                                start=(ha == 0), stop=(ha == H - 1))
                    nc.scalar.copy(htS[:, cc, :], ht_ps[:])

            # normalizer q (applied after s4's c-contraction)
            with tc.tile_pool(name="nrm", bufs=1, space="PSUM") as psN, \
                 tc.tile_pool(name="nrmt", bufs=1) as pnt:
                psR = psN.tile([1, P], F32, tag="rowsum")
                for cc in range(CC):
                    nc.tensor.matmul(psR[:], lhsT=ones128r[:],
                                     rhs=htS[:, cc, :],
                                     start=(cc == 0), stop=(cc == CC - 1))
                q = pnt.tile([1, P], F32, tag="q")
                nc.vector.tensor_scalar_add(q[:], psR[:], 108.0e-5)
                qr = pnt.tile([1, P], F32, tag="qr")
                nc.vector.reciprocal(qr[:], q[:])
                psQ = psN.tile([128, P], F32, tag="qb")
                nc.tensor.matmul(psQ[:], lhsT=ones1[:], rhs=qr[:],
                                 start=True, stop=True)
                nc.scalar.copy(qb[:], psQ[:])

            # ---- stage 4: rs^T = (seq^T @ htS) * q ----
            with tc.tile_pool(name="seqp", bufs=1) as psq, \
                 tc.tile_pool(name="rsps", bufs=2, space="PSUM") as psRS:
                seq_sb = psq.tile([128, CC, D], F32R)
                nc.sync.dma_start(seq_sb[:], seq_p.rearrange("p (c d) -> p c d", c=CC))
                for mc in range(DC):
                    ps = psRS.tile([128, P], F32, tag="rs")
                    for cc in range(CC):
                        nc.tensor.matmul(
                            ps[:],
                            lhsT=seq_sb[:, cc, mc * 128:(mc + 1) * 128],
                            rhs=htS[:, cc, :],
                            start=(cc == 0), stop=(cc == CC - 1))
                    nc.vector.tensor_mul(rsT[:, mc, :], ps[:], qb[:])
            ps34_cm.__exit__(None, None, None)
            psel_cm.__exit__(None, None, None)

            # ---- stages 5+6 interleaved over mc / k ----
            dve_units = NJ - GP_UNITS  # per k, from unit 0
            psL_cm = tc.tile_pool(name="lg", bufs=1, space="PSUM")
            psL = psL_cm.__enter__()
            logits_ps = psL.tile([NREL, P], F32)
            with tc.tile_pool(name="wexp", bufs=2) as pwx, \
                 tc.tile_pool(name="wbs", bufs=2) as pwb, \
                 tc.tile_pool(name="ztr", bufs=2) as pzt, \
                 tc.tile_pool(name="zhc", bufs=2) as pzh, \
                 tc.tile_pool(name="otp", bufs=1) as pot, \
                 tc.tile_pool(name="exps", bufs=2, space="PSUM") as psE, \
                 tc.tile_pool(name="zrp", bufs=1, space="PSUM") as psZ:
                for mc in range(DC):
                    # s5 chunk mc: zh then zt
                    wh_mc = pwx.tile([128, KC, 128], F32R, tag="whm")
                    nc.sync.dma_start(wh_mc[:], Whm[:, mc].rearrange(
                        "p (k j) -> p k j", k=KC))
                    wt_mc = pwx.tile([128, KC, 128], F32R, tag="wtm")
                    nc.sync.dma_start(wt_mc[:], Wtm[:, mc].rearrange(
                        "p (k j) -> p k j", k=KC))
                    ps = psE.tile([128, P], F32, tag="ex")
                    for kc in range(KC):
                        rhs = hsT[:, kc, :] if kc < DC else rsT[:, kc - DC, :]
                        nc.tensor.matmul(
                            ps[:], lhsT=wh_mc[:, kc, :],
                            rhs=rhs, start=(kc == 0), stop=(kc == KC - 1))
                    nc.scalar.activation(zhT[:, mc, :], ps[:], AF.Tanh,
                                         bias=bh_sb[:, mc:mc + 1])
                    nc.scalar.activation(zhT[:, mc, :], zhT[:, mc, :], AF.Tanh)
                    psb = psE.tile([128, P], F32, tag="ex")
                    for kc in range(KC):
                        rhs = tsT[:, kc, :] if kc < DC else rsT[:, kc - DC, :]
                        nc.tensor.matmul(
                            psb[:], lhsT=wt_mc[:, kc, :],
                            rhs=rhs, start=(kc == 0), stop=(kc == KC - 1))
                    nc.scalar.activation(ztT[:, mc, :], psb[:], AF.Tanh,
                                         bias=bt_sb[:, mc:mc + 1])

                    # s6 for k = 2mc, 2mc+1 (batched per mc)
                    ks = (2 * mc, 2 * mc + 1)
                    ztreps, zhcs_k = {}, {}
                    for k in ks:
                        kc2, hf = k // 2, k % 2
                        p0 = hf * 64
                        ztrep = pzt.tile([128, CC, P], F32R, tag="ztr")
                        for ch in range(8):
                            nc.sync.dma_start(
                                ztrep[0:8, ch, :],
                                ztT[p0 + ch * 8:p0 + ch * 8 + 8, kc2, :])
                        s = 8
                        while s < 128:
                            nc.sync.dma_start(ztrep[s:2 * s, :, :],
                                              ztrep[0:s, :, :])
                            s *= 2
                        ztreps[k] = ztrep
                    # zh-rep MMs + copies for both k (bh3 first: feeds gpsimd)
                    for k in ks:
                        kc2, hf = k // 2, k % 2
                        p0 = hf * 64
                        zhcs = [None] * 4
                        for bh2 in (3, 0, 1, 2):
                            zr = psZ.tile([128, P], F32, tag=f"zr{bh2}")
                            nc.tensor.matmul(
                                zr[:], lhsT=selz_sb[p0:p0 + 64, bh2, :],
                                rhs=zhT[p0:p0 + 64, kc2, :],
                                start=True, stop=True)
                            zc = pzh.tile([128, P], F32R, tag=f"zhc{bh2}")
                            nc.scalar.copy(zc[:], zr[:])
                            zhcs[bh2] = zc
                        zhcs_k[k] = zhcs
                    # products + bilinear per k (GP op emitted first)
                    for k in ks:
                        ztrep = ztreps[k]
                        zhcs = zhcs_k[k]
                        o_ts = [None] * 4
                        for bh2 in (3, 0, 1, 2):
                            zcb = zhcs[bh2][:, :].rearrange(
                                "p (o n) -> p o n", o=1)
                            o_t = pot.tile([128, 8, P], F32R, tag=f"ot{bh2}")
                            eng = nc.gpsimd if bh2 == 3 else nc.vector
                            eng.tensor_tensor(
                                o_t[:, :, :], ztrep[:, 0:8, :],
                                zcb.to_broadcast([128, 8, P]),
                                op=OP.mult)
                            o_ts[bh2] = o_t
                        wb_k = pwb.tile([128, NJ, NREL], F32R, tag="wbk")
                        nc.sync.dma_start(
                            wb_k[:], Wb[k].rearrange("p (j r) -> p j r", j=NJ))
                        for bh2 in range(4):
                            for ch in range(8):
                                j = bh2 * 8 + ch
                                nc.tensor.matmul(
                                    logits_ps[:],
                                    lhsT=wb_k[:, j, :],
                                    rhs=o_ts[bh2][:, ch, :],
                                    start=(k == 0 and j == 0),
                                    stop=(k == KB - 1 and j == NJ - 1))

            # ---- stage 7: bias, preds, write out ----
            with tc.tile_pool(name="fin", bufs=1, space="PSUM") as psF:
                nc.vector.tensor_scalar_add(logits_sb[:], logits_ps[:],
                                            bb_sb[:, 0:1])
                nc.sync.dma_start(logitsT_out[:, :], logits_sb[:])
                psTh = psF.tile([NREL, P], F32, tag="th")
                nc.tensor.matmul(psTh[:], lhsT=ones1r[:],
                                 rhs=logits_sb[0:1, :], start=True, stop=True)
                nc.vector.tensor_tensor(preds_sb[:], logits_sb[:], psTh[:],
                                        op=OP.is_gt)
                psCt = psF.tile([1, P], F32, tag="cnt")
                nc.tensor.matmul(psCt[:], lhsT=ones97[:],
                                 rhs=preds_sb[:, :], start=True, stop=True)
                nc.vector.tensor_single_scalar(preds_sb[0:1, :], psCt[:],
                                               0.0, OP.is_equal)
                nc.sync.dma_start(predsT_out[:, :], preds_sb[:])
            psL_cm.__exit__(None, None, None)
            pact_cm.__exit__(None, None, None)

    if split_waits:
        _split_multiwaits(nc)
    nc.finalize()
    return nc


def _permute_wb(Wb):
    """Wb rows (k, b, c) -> [k, part=b16*8+c8, j=(bh*8+ch), r] with
    b = bh*16+b16, c = ch*8+c8."""
    W = np.asarray(Wb, np.float32).reshape(KB, 4, 16, 8, 8, NREL)
    W = W.transpose(0, 2, 4, 1, 3, 5)            # k, b16, c8, bh, ch, r
    return np.ascontiguousarray(W.reshape(KB, 128, NJ * NREL))


def _permute_wx(W):
    """[1536, 768] -> [128, mc, kc*128] column-chunked for streaming."""
    W = np.asarray(W, np.float32).reshape(KC, 128, DC, 128)
    W = W.transpose(1, 2, 0, 3)                  # p, mc, kc, j
    return np.ascontiguousarray(W.reshape(128, DC, KC * 128))


def _make_selz():
    s = np.zeros((128, 4, 128), np.float32)
    for bh2 in range(4):
        for part in range(128):
            s[bh2 * 16 + part // 8, bh2, part] = 1.0          # half 0 rows
            s[64 + bh2 * 16 + part // 8, bh2, part] = 1.0     # half 1 rows
    return np.ascontiguousarray(s.reshape(128, 4 * 128))


def _make_inputs(core, sequence_output, attention, mention_pos, hts,
                 Wh, bh, Wt, bt, Wb, bb):
    d = core // 2
    half = core % 2
    pos = (np.asarray(mention_pos[d]) + 1).astype(np.int32)      # [E, M]
    ht = np.asarray(hts[d][half * P:(half + 1) * P]).astype(np.int64)  # [P,2]
    h_idx, t_idx = ht[:, 0], ht[:, 1]

    hsel1 = np.zeros((E, P), np.float32)
    hsel1[h_idx, np.arange(P)] = 1.0
    tsel1 = np.zeros((E, P), np.float32)
    tsel1[t_idx, np.arange(P)] = 1.0
    hsel3 = np.zeros((E * M, P), np.float32)
    tsel3 = np.zeros((E * M, P), np.float32)
    for m in range(M):
        hsel3[h_idx * M + m, np.arange(P)] = 1.0
        tsel3[t_idx * M + m, np.arange(P)] = 1.0

    seq_d = np.asarray(sequence_output[d], np.float32)
    im = {
        "seq": np.ascontiguousarray(seq_d),
        "seq_p": np.ascontiguousarray(
            seq_d.reshape(CC, 128, D).transpose(1, 0, 2).reshape(128, CC * D)),
        "posm": pos,
        "posf": np.ascontiguousarray(pos.reshape(E * M, 1)),
        "hsel1": hsel1, "tsel1": tsel1, "hsel3": hsel3, "tsel3": tsel3,
        "Whm": _CACHE.setdefault("Whm", _permute_wx(Wh)),
        "Wtm": _CACHE.setdefault("Wtm", _permute_wx(Wt)),
        "Wb": _CACHE.setdefault("Wbp", _permute_wb(Wb)),
        "bh": np.ascontiguousarray(bh, np.float32),
        "bt": np.ascontiguousarray(bt, np.float32),
        "bb": np.ascontiguousarray(bb, np.float32),
        "ident": _CACHE.setdefault(
            "ident", np.ascontiguousarray(np.eye(128, dtype=np.float32))),
        "onesr": _CACHE.setdefault(
            "onesr", np.ones((128, 1), np.float32)),
        "selz": _CACHE.setdefault("selz", _make_selz()),
    }
    for h in range(H):
        im[f"att{h}"] = np.ascontiguousarray(attention[d, h], np.float32)
    return im


LAST_RESULTS = None


def kernel(sequence_output, attention, mention_pos, hts,
           Wh, bh, Wt, bt, Wb, bb):
    global LAST_RESULTS
    if "nc" not in _CACHE:
        _CACHE["nc"] = build_nc()
    nc = _CACHE["nc"]

    in_maps = [_make_inputs(c, sequence_output, attention, mention_pos, hts,
                            Wh, bh, Wt, bt, Wb, bb) for c in range(NCORES)]
    res = run_bass_kernel_spmd(nc, in_maps, core_ids=list(range(NCORES)))
    LAST_RESULTS = res

    logits = np.concatenate(
        [np.ascontiguousarray(r["logitsT"].T) for r in res.results], axis=0)
    preds = np.concatenate(
        [np.ascontiguousarray(r["predsT"].T) for r in res.results], axis=0)
    return logits.astype(np.float32), preds.astype(np.float32)
